# revision 19
# baseline (speedup 1.0000x reference)
"""Trainium2 Bass kernel for upsample_conv_2d (conv_transpose stride-2 3x3 +
4x4 FIR + bias), data-parallel over batch on 8 NeuronCores.

Formulation (2x less PE work than composing conv+FIR into one 6x6 kernel,
and vector work small enough to hide under the PE):

stage 1: The conv_transpose phase images combined with the W-DIRECTION half
of the separable 4x4 FIR (fir = outer([1,3,3,1],[1,3,3,1])/16) collapse into
18 composed channel-contraction taps: for output column parity pw and row
set a (a=0: even y-rows from w rows p=0,2; a=1: odd y-rows from p=1),

    E[a,pw][i,S] = sum_{d,o} CW[a,pw,d,o] . xp[i+d, S+o]   (+ bias/8)

with CW 3 column-taps (o) per row-tap (d); a=0 has d in {0,1}, a=1 only d=1.
All taps run as bf16 matmuls accumulating in fp32 PSUM; the PSUM drain adds
bias/8 (so the 8-coefficient pass-h sum reconstructs + bias exactly) and
writes bf16 E tiles. E[a=1] is stored with pad rows 0,65 holding bias/8.

stage 2 (pass-h only): out rows by parity pa combine two E images with
integer weights (1/16 already folded into CW):

    out[2R+pa, *] = 3*E0[R+pa] + E0[R+1-pa] + 3*F1[R+1] + F1[R+2*pa]

as two whole-image scalar_tensor_tensor ops (DVE, bf16 aligned fast path)
plus one tensor_tensor add per 16-row block writing the strided f32 output
tile directly (split DVE/GpSimd). Output DMA moves flat [128 x 16KB]
contiguous blocks (out DRAM tensor declared layout-equivalent
[2,128,4,4096]).
"""

import json

import numpy as np
import ml_dtypes

import concourse.bass as bass
import concourse.mybir as mybir
import concourse.tile as tile
from concourse.bass_utils import run_bass_kernel_spmd

# ---------------------------------------------------------------------------
# BIR post-pass: this walrus build rejects instructions carrying more than one
# sem wait (e.g. Tile's kernel-tail Drain gets 3). Hoist extras into
# standalone EventSemaphore instructions right before the owner.
# ---------------------------------------------------------------------------
_MAX_WAITS = 1


def _split_waits(j: dict) -> dict:
    for fn in j.get("functions", []):
        for blk in fn.get("blocks", []):
            insts = blk.get("instructions")
            if not insts:
                continue
            out = []
            for inst in insts:
                si = inst.get("sync_info") or {}
                waits = si.get("on_wait") or []
                if len(waits) > _MAX_WAITS:
                    for k, w in enumerate(waits[_MAX_WAITS:]):
                        out.append(
                            {
                                "debug": inst.get("debug", 0),
                                "engine": inst["engine"],
                                "ins": [],
                                "name": f"{inst['name']}-wsplit{k}",
                                "opcode": "EventSemaphore",
                                "outs": [],
                                "sync_info": {"on_update": [], "on_wait": [w]},
                            }
                        )
                    si["on_wait"] = waits[:_MAX_WAITS]
                out.append(inst)
            blk["instructions"] = out
    return j


_orig_to_json_bytes = bass.Bass.to_json_bytes


def _patched_to_json_bytes(self):
    return json.dumps(_split_waits(json.loads(_orig_to_json_bytes(self)))).encode()


bass.Bass.to_json_bytes = _patched_to_json_bytes

# ---------------------------------------------------------------------------
# Problem constants (hardcoded; kernel.py must be self-contained)
# ---------------------------------------------------------------------------
N, C, H, W = 8, 256, 64, 64
OH, OW = 2 * H, 2 * W
N_CORES = 8
F32 = mybir.dt.float32
BF16 = mybir.dt.bfloat16
AOP = mybir.AluOpType
IDENT = mybir.ActivationFunctionType.Identity

# stage-1 composed tap schedule: per pw, rows (a=0,d=0,p=0), (a=0,d=1,p=2),
# (a=1,d=1,p=1), each with col offsets o=0,1,2. t = pw*9 + row*3 + o.
_ROWS = ((0, 0, 0), (0, 1, 2), (1, 1, 1))  # (a, d, p)
_NTAP = 18


def _stage1_weight_matrix(w: np.ndarray) -> np.ndarray:
    """[256,256,3,3] weight -> [128, 72*128] lhsT matrix (f32, cast later).

    Column block index = (t*2 + cib)*2 + cob; block = CW[t][cob128, cib128].T.
    CW composes the w-direction FIR half into the conv taps, including the
    1/16 FIR normalization.
    """
    Wq = {(p, q): w[:, :, p, q].astype(np.float64) for p in range(3) for q in range(3)}
    mats = []
    for pw in range(2):
        for a, d, p in _ROWS:
            if pw == 0:
                cc = {
                    0: 3 * Wq[p, 0] + Wq[p, 1],
                    1: 3 * Wq[p, 2] + Wq[p, 0] + 3 * Wq[p, 1],
                    2: Wq[p, 2],
                }
            else:
                cc = {
                    0: Wq[p, 0],
                    1: 3 * Wq[p, 0] + Wq[p, 2] + 3 * Wq[p, 1],
                    2: 3 * Wq[p, 2] + Wq[p, 1],
                }
            for o in range(3):
                mats.append(cc[o] / 16.0)
    Wmat = np.zeros((128, _NTAP * 4, 128), dtype=np.float32)
    for t, M in enumerate(mats):
        for cib in range(2):
            for cob in range(2):
                blk = M[cob * 128 : (cob + 1) * 128, cib * 128 : (cib + 1) * 128]
                Wmat[:, (t * 2 + cib) * 2 + cob, :] = blk.T.astype(np.float32)
    return Wmat.reshape(128, -1)


def _tap_index(t: int, cib: int, cob: int) -> int:
    return (t * 2 + cib) * 2 + cob


def _blocks(nrows: int):
    out = []
    r = 0
    while r < nrows:
        nr = min(8, nrows - r)  # 8 rows x 64 = 512 fp32 = exactly one PSUM bank
        out.append((r, nr))
        r += nr
    return out


def build_nc(reps: int = 1) -> bass.Bass:
    nc = bass.Bass("TRN2", target_bir_lowering=False, debug=False)
    x_d = nc.dram_tensor("x", [C, H + 2, W + 2], BF16, kind="ExternalInput").ap()
    w_d = nc.dram_tensor("w", [128, _NTAP * 4 * 128], BF16, kind="ExternalInput").ap()
    b_d = nc.dram_tensor("bias8", [2, 128], F32, kind="ExternalInput").ap()
    # layout-equivalent view of [C, OH, OW]: [cob, ch, hb2, 32*OW]
    out_d = nc.dram_tensor("out", [2, 128, 4, 32 * OW], F32, kind="ExternalOutput").ap()

    xb = x_d.rearrange("(b p) h w -> b p h w", p=128)

    with tile.TileContext(nc) as tc:
        with (
            tc.tile_pool(name="weights", bufs=1) as wpool,
            tc.tile_pool(name="xin", bufs=1) as xpool,
            tc.tile_pool(name="ebuf", bufs=1) as epool,
            tc.tile_pool(name="psum", bufs=8, space="PSUM") as ppool,
            tc.tile_pool(name="outs", bufs=2) as opool,
            tc.tile_pool(name="scratch", bufs=2) as spool,
        ):
            wt = wpool.tile([128, _NTAP * 4, 128], BF16)
            nc.sync.dma_start(wt[:], w_d.rearrange("p (a b) -> p a b", b=128))
            bt8 = wpool.tile([128, 2], F32)
            nc.sync.dma_start(bt8[:], b_d.rearrange("b p -> p b"))

            xpad = [
                xpool.tile([128, H + 2, W + 2], BF16, tag=f"xp{i}", name=f"xp{i}")
                for i in range(2)
            ]
            for cib in range(2):
                nc.sync.dma_start(xpad[cib][:], xb[cib])

            # E tiles: E0 [65,64]; F1 [66,64] with pad rows 0,65 = bias/8
            te = {}
            for cob in range(2):
                for pw in range(2):
                    te[0, pw, cob] = epool.tile(
                        [128, 65, 64], BF16, tag=f"e0{pw}_{cob}", name=f"e0{pw}_{cob}"
                    )
                    te[1, pw, cob] = epool.tile(
                        [128, 66, 64], BF16, tag=f"e1{pw}_{cob}", name=f"e1{pw}_{cob}"
                    )
            # fill F1 pad rows with bias/8 once (out = in*0 + bias)
            for cob in range(2):
                for pw in range(2):
                    for rr in (0, 65):
                        nc.scalar.activation(
                            te[1, pw, cob][:, rr : rr + 1, :],
                            xpad[0][:, 0:1, 0:64],
                            IDENT,
                            bias=bt8[:, cob : cob + 1],
                            scale=0.0,
                        )

            def drain(dst_ap, psum_ap, cob):
                # ACT only: keeps DVE free for the pass-h stt ops
                nc.scalar.activation(
                    dst_ap, psum_ap, IDENT, bias=bt8[:, cob : cob + 1], scale=1.0
                )

            fcnt = 0
            for _rep in range(reps):
                for cob in range(2):
                    # ---- stage 1: E images via composed-tap PE matmuls ----
                    for pw in range(2):
                        for ri, (a, d, _p) in enumerate(_ROWS):
                            et = te[a, pw, cob]
                            nrows = 65 if a == 0 else 64
                            # a=0 accumulates rows (0,0,0) and (0,1,2) into one
                            # psum pass; handled below by grouping on a.
                            if ri == 1:
                                continue  # merged into ri == 0 pass
                            if a == 0:
                                row_list = [_ROWS[0], _ROWS[1]]
                            else:
                                row_list = [_ROWS[2]]
                            for r0, nr in _blocks(nrows):
                                ps = ppool.tile([128, nr, 64], F32, tag="ps", name="ps")
                                nmm = len(row_list) * 3 * 2
                                i = 0
                                for rj, (aa, dd, _pp) in enumerate(row_list):
                                    tbase = pw * 9 + (_ROWS.index((aa, dd, _pp))) * 3
                                    for o in range(3):
                                        for cib in range(2):
                                            lhsT = wt[:, _tap_index(tbase + o, cib, cob), :]
                                            rhs = xpad[cib][
                                                :, r0 + dd : r0 + dd + nr, o : o + 64
                                            ]
                                            nc.tensor.matmul(
                                                ps[:],
                                                lhsT,
                                                rhs,
                                                start=(i == 0),
                                                stop=(i == nmm - 1),
                                            )
                                            i += 1
                                dst = (
                                    et[:, r0 : r0 + nr, :]
                                    if a == 0
                                    else et[:, r0 + 1 : r0 + 1 + nr, :]
                                )
                                drain(dst, ps[:], cob)

                    # ---- stage 2: pass-h in 32-R-row halves ----
                    for half in range(2):
                        r0 = half * 32
                        tsc = {}
                        for pw in range(2):
                            e0 = te[0, pw, cob]
                            f1 = te[1, pw, cob]
                            for pa in range(2):
                                t1 = spool.tile(
                                    [128, 32, 64], BF16,
                                    tag=f"t1_{pw}{pa}", name="t1",
                                )
                                t2 = spool.tile(
                                    [128, 32, 64], BF16,
                                    tag=f"t2_{pw}{pa}", name="t2",
                                )
                                e0_3 = e0[:, r0 + pa : r0 + pa + 32, :]
                                e0_1 = e0[:, r0 + 1 - pa : r0 + 33 - pa, :]
                                f1_3 = f1[:, r0 + 1 : r0 + 33, :]
                                f1_1 = f1[:, r0 + 2 * pa : r0 + 2 * pa + 32, :]
                                nc.vector.scalar_tensor_tensor(
                                    t1[:], e0_3, 3.0, e0_1, AOP.mult, AOP.add
                                )
                                nc.vector.scalar_tensor_tensor(
                                    t2[:], f1_3, 3.0, f1_1, AOP.mult, AOP.add
                                )
                                tsc[pw, pa] = (t1, t2)
                        for sub in range(2):
                            hb2 = half * 2 + sub
                            lo = sub * 16
                            ot = opool.tile(
                                [128, 16, 2, 64, 2], F32, tag="ot", name="ot"
                            )
                            for pw in range(2):
                                for pa in range(2):
                                    t1, t2 = tsc[pw, pa]
                                    # mostly GpSimd; DVE (busy with stt) takes
                                    # 1 in 8 to equalize engine time
                                    eng = nc.vector if fcnt % 8 == 0 else nc.gpsimd
                                    fcnt += 1
                                    eng.tensor_tensor(
                                        ot[:, :, pa, :, pw],
                                        t1[:, lo : lo + 16, :],
                                        t2[:, lo : lo + 16, :],
                                        AOP.add,
                                    )
                            nc.sync.dma_start(
                                out_d[cob, :, hb2, :],
                                ot[:].rearrange("c a b w v -> c (a b w v)"),
                            )
    return nc


_CACHED_NC = {}


def _get_nc(reps: int = 1) -> bass.Bass:
    if reps not in _CACHED_NC:
        _CACHED_NC[reps] = build_nc(reps)
    return _CACHED_NC[reps]


def _prepare(x, weight, bias, reps: int = 1):
    Wmat = _stage1_weight_matrix(np.asarray(weight, dtype=np.float32)).astype(
        ml_dtypes.bfloat16
    )
    b2 = np.ascontiguousarray(
        np.asarray(bias, dtype=np.float32).reshape(2, 128) / np.float32(8.0)
    )
    xs = np.pad(
        np.asarray(x, dtype=np.float32), ((0, 0), (0, 0), (1, 1), (1, 1))
    ).astype(ml_dtypes.bfloat16)
    nc = _get_nc(reps)
    in_maps = [{"x": xs[i], "w": Wmat, "bias8": b2} for i in range(N_CORES)]
    return in_maps, nc


def _run(x, weight, bias, reps: int = 1):
    in_maps, nc = _prepare(x, weight, bias, reps)
    res = run_bass_kernel_spmd(nc, in_maps, list(range(N_CORES)))
    return np.stack(
        [res.results[i]["out"].reshape(C, OH, OW) for i in range(N_CORES)]
    )


def kernel(x, weight, bias):
    return _run(x, weight, bias, reps=1)


# revision 21
# speedup vs baseline: 1.0095x; 1.0095x over previous
"""Trainium2 Bass kernel for upsample_conv_2d (conv_transpose stride-2 3x3 +
4x4 FIR + bias), data-parallel over batch on 8 NeuronCores.

Formulation (2x less PE work than composing conv+FIR into one 6x6 kernel,
and vector work small enough to hide under the PE):

stage 1: The conv_transpose phase images combined with the W-DIRECTION half
of the separable 4x4 FIR (fir = outer([1,3,3,1],[1,3,3,1])/16) collapse into
18 composed channel-contraction taps: for output column parity pw and row
set a (a=0: even y-rows from w rows p=0,2; a=1: odd y-rows from p=1),

    E[a,pw][i,S] = sum_{d,o} CW[a,pw,d,o] . xp[i+d, S+o]   (+ bias/8)

with CW 3 column-taps (o) per row-tap (d); a=0 has d in {0,1}, a=1 only d=1.
All taps run as bf16 matmuls accumulating in fp32 PSUM; the PSUM drain adds
bias/8 (so the 8-coefficient pass-h sum reconstructs + bias exactly) and
writes bf16 E tiles. E[a=1] is stored with pad rows 0,65 holding bias/8.

stage 2 (pass-h only): out rows by parity pa combine two E images with
integer weights (1/16 already folded into CW):

    out[2R+pa, *] = 3*E0[R+pa] + E0[R+1-pa] + 3*F1[R+1] + F1[R+2*pa]

as two whole-image scalar_tensor_tensor ops (DVE, bf16 aligned fast path)
plus one tensor_tensor add per 16-row block writing the strided f32 output
tile directly (split DVE/GpSimd). Output DMA moves flat [128 x 16KB]
contiguous blocks (out DRAM tensor declared layout-equivalent
[2,128,4,4096]).
"""

import json

import numpy as np
import ml_dtypes

import concourse.bass as bass
import concourse.mybir as mybir
import concourse.tile as tile
from concourse.bass_utils import run_bass_kernel_spmd

# ---------------------------------------------------------------------------
# BIR post-pass: this walrus build rejects instructions carrying more than one
# sem wait (e.g. Tile's kernel-tail Drain gets 3). Hoist extras into
# standalone EventSemaphore instructions right before the owner.
# ---------------------------------------------------------------------------
_MAX_WAITS = 1


def _split_waits(j: dict) -> dict:
    for fn in j.get("functions", []):
        for blk in fn.get("blocks", []):
            insts = blk.get("instructions")
            if not insts:
                continue
            out = []
            for inst in insts:
                si = inst.get("sync_info") or {}
                waits = si.get("on_wait") or []
                if len(waits) > _MAX_WAITS:
                    for k, w in enumerate(waits[_MAX_WAITS:]):
                        out.append(
                            {
                                "debug": inst.get("debug", 0),
                                "engine": inst["engine"],
                                "ins": [],
                                "name": f"{inst['name']}-wsplit{k}",
                                "opcode": "EventSemaphore",
                                "outs": [],
                                "sync_info": {"on_update": [], "on_wait": [w]},
                            }
                        )
                    si["on_wait"] = waits[:_MAX_WAITS]
                out.append(inst)
            blk["instructions"] = out
    return j


_orig_to_json_bytes = bass.Bass.to_json_bytes


def _patched_to_json_bytes(self):
    return json.dumps(_split_waits(json.loads(_orig_to_json_bytes(self)))).encode()


bass.Bass.to_json_bytes = _patched_to_json_bytes

# ---------------------------------------------------------------------------
# Problem constants (hardcoded; kernel.py must be self-contained)
# ---------------------------------------------------------------------------
N, C, H, W = 8, 256, 64, 64
OH, OW = 2 * H, 2 * W
N_CORES = 8
F32 = mybir.dt.float32
BF16 = mybir.dt.bfloat16
AOP = mybir.AluOpType
IDENT = mybir.ActivationFunctionType.Identity

# stage-1 composed tap schedule: per pw, rows (a=0,d=0,p=0), (a=0,d=1,p=2),
# (a=1,d=1,p=1), each with col offsets o=0,1,2. t = pw*9 + row*3 + o.
_ROWS = ((0, 0, 0), (0, 1, 2), (1, 1, 1))  # (a, d, p)
_NTAP = 18


def _stage1_weight_matrix(w: np.ndarray) -> np.ndarray:
    """[256,256,3,3] weight -> [128, 72*128] lhsT matrix (f32, cast later).

    Column block index = (t*2 + cib)*2 + cob; block = CW[t][cob128, cib128].T.
    CW composes the w-direction FIR half into the conv taps, including the
    1/16 FIR normalization.
    """
    Wq = {(p, q): w[:, :, p, q].astype(np.float64) for p in range(3) for q in range(3)}
    mats = []
    for pw in range(2):
        for a, d, p in _ROWS:
            if pw == 0:
                cc = {
                    0: 3 * Wq[p, 0] + Wq[p, 1],
                    1: 3 * Wq[p, 2] + Wq[p, 0] + 3 * Wq[p, 1],
                    2: Wq[p, 2],
                }
            else:
                cc = {
                    0: Wq[p, 0],
                    1: 3 * Wq[p, 0] + Wq[p, 2] + 3 * Wq[p, 1],
                    2: 3 * Wq[p, 2] + Wq[p, 1],
                }
            for o in range(3):
                mats.append(cc[o] / 16.0)
    Wmat = np.zeros((128, _NTAP * 4, 128), dtype=np.float32)
    for t, M in enumerate(mats):
        for cib in range(2):
            for cob in range(2):
                blk = M[cob * 128 : (cob + 1) * 128, cib * 128 : (cib + 1) * 128]
                Wmat[:, (t * 2 + cib) * 2 + cob, :] = blk.T.astype(np.float32)
    return Wmat.reshape(128, -1)


def _tap_index(t: int, cib: int, cob: int) -> int:
    return (t * 2 + cib) * 2 + cob


def _blocks(nrows: int):
    out = []
    r = 0
    while r < nrows:
        nr = min(8, nrows - r)  # 8 rows x 64 = 512 fp32 = exactly one PSUM bank
        out.append((r, nr))
        r += nr
    return out


def build_nc(reps: int = 1) -> bass.Bass:
    nc = bass.Bass("TRN2", target_bir_lowering=False, debug=False)
    x_d = nc.dram_tensor("x", [C, H + 2, W + 2], BF16, kind="ExternalInput").ap()
    w_d = nc.dram_tensor("w", [128, _NTAP * 4 * 128], BF16, kind="ExternalInput").ap()
    b_d = nc.dram_tensor("bias8", [2, 128], F32, kind="ExternalInput").ap()
    # layout-equivalent view of [C, OH, OW]: [cob, ch, hb2, 32*OW]
    out_d = nc.dram_tensor("out", [2, 128, 4, 32 * OW], F32, kind="ExternalOutput").ap()

    xb = x_d.rearrange("(b p) h w -> b p h w", p=128)

    with tile.TileContext(nc) as tc:
        with (
            tc.tile_pool(name="weights", bufs=1) as wpool,
            tc.tile_pool(name="xin", bufs=1) as xpool,
            tc.tile_pool(name="ebuf", bufs=1) as epool,
            tc.tile_pool(name="psum", bufs=8, space="PSUM") as ppool,
            tc.tile_pool(name="outs", bufs=2) as opool,
            tc.tile_pool(name="scratch", bufs=2) as spool,
        ):
            wt = wpool.tile([128, _NTAP * 4, 128], BF16)
            nc.sync.dma_start(wt[:], w_d.rearrange("p (a b) -> p a b", b=128))
            bt8 = wpool.tile([128, 2], F32)
            nc.sync.dma_start(bt8[:], b_d.rearrange("b p -> p b"))

            xpad = [
                xpool.tile([128, H + 2, W + 2], BF16, tag=f"xp{i}", name=f"xp{i}")
                for i in range(2)
            ]
            for cib in range(2):
                nc.sync.dma_start(xpad[cib][:], xb[cib])

            # E tiles: E0 [65,64]; F1 [66,64] with pad rows 0,65 = bias/8
            te = {}
            for cob in range(2):
                for pw in range(2):
                    te[0, pw, cob] = epool.tile(
                        [128, 65, 64], BF16, tag=f"e0{pw}_{cob}", name=f"e0{pw}_{cob}"
                    )
                    te[1, pw, cob] = epool.tile(
                        [128, 66, 64], BF16, tag=f"e1{pw}_{cob}", name=f"e1{pw}_{cob}"
                    )
            # fill F1 pad rows with bias/8 once (out = in*0 + bias)
            for cob in range(2):
                for pw in range(2):
                    for rr in (0, 65):
                        nc.scalar.activation(
                            te[1, pw, cob][:, rr : rr + 1, :],
                            xpad[0][:, 0:1, 0:64],
                            IDENT,
                            bias=bt8[:, cob : cob + 1],
                            scale=0.0,
                        )

            def drain(dst_ap, psum_ap, cob):
                # ACT only: keeps DVE free for the pass-h stt ops
                nc.scalar.activation(
                    dst_ap, psum_ap, IDENT, bias=bt8[:, cob : cob + 1], scale=1.0
                )

            fcnt = 0
            for _rep in range(reps):
                for cob in range(2):
                    # ---- stage 1: E images via composed-tap PE matmuls ----
                    for pw in range(2):
                        for ri, (a, d, _p) in enumerate(_ROWS):
                            et = te[a, pw, cob]
                            nrows = 65 if a == 0 else 64
                            # a=0 accumulates rows (0,0,0) and (0,1,2) into one
                            # psum pass; handled below by grouping on a.
                            if ri == 1:
                                continue  # merged into ri == 0 pass
                            if a == 0:
                                row_list = [_ROWS[0], _ROWS[1]]
                            else:
                                row_list = [_ROWS[2]]
                            for r0, nr in _blocks(nrows):
                                ps = ppool.tile([128, nr, 64], F32, tag="ps", name="ps")
                                nmm = len(row_list) * 3 * 2
                                i = 0
                                for rj, (aa, dd, _pp) in enumerate(row_list):
                                    tbase = pw * 9 + (_ROWS.index((aa, dd, _pp))) * 3
                                    for o in range(3):
                                        for cib in range(2):
                                            lhsT = wt[:, _tap_index(tbase + o, cib, cob), :]
                                            rhs = xpad[cib][
                                                :, r0 + dd : r0 + dd + nr, o : o + 64
                                            ]
                                            nc.tensor.matmul(
                                                ps[:],
                                                lhsT,
                                                rhs,
                                                start=(i == 0),
                                                stop=(i == nmm - 1),
                                            )
                                            i += 1
                                dst = (
                                    et[:, r0 : r0 + nr, :]
                                    if a == 0
                                    else et[:, r0 + 1 : r0 + 1 + nr, :]
                                )
                                drain(dst, ps[:], cob)

                    # ---- stage 2: pass-h in 32-R-row halves ----
                    for half in range(2):
                        r0 = half * 32
                        tsc = {}
                        for pw in range(2):
                            e0 = te[0, pw, cob]
                            f1 = te[1, pw, cob]
                            for pa in range(2):
                                t1 = spool.tile(
                                    [128, 32, 64], BF16,
                                    tag=f"t1_{pw}{pa}", name="t1",
                                )
                                t2 = spool.tile(
                                    [128, 32, 64], BF16,
                                    tag=f"t2_{pw}{pa}", name="t2",
                                )
                                e0_3 = e0[:, r0 + pa : r0 + pa + 32, :]
                                e0_1 = e0[:, r0 + 1 - pa : r0 + 33 - pa, :]
                                f1_3 = f1[:, r0 + 1 : r0 + 33, :]
                                f1_1 = f1[:, r0 + 2 * pa : r0 + 2 * pa + 32, :]
                                nc.vector.scalar_tensor_tensor(
                                    t1[:], e0_3, 3.0, e0_1, AOP.mult, AOP.add
                                )
                                nc.vector.scalar_tensor_tensor(
                                    t2[:], f1_3, 3.0, f1_1, AOP.mult, AOP.add
                                )
                                # sum into t1 in place on GpSimd (contiguous
                                # bf16; aliasing is free)
                                nc.gpsimd.tensor_tensor(
                                    t1[:], t1[:], t2[:], AOP.add
                                )
                                tsc[pw, pa] = t1
                        for sub in range(2):
                            hb2 = half * 2 + sub
                            lo = sub * 16
                            ot = opool.tile(
                                [128, 16, 2, 64, 2], F32, tag="ot", name="ot"
                            )
                            for pw in range(2):
                                for pa in range(2):
                                    t1 = tsc[pw, pa]
                                    # ACT does the strided f32 interleave
                                    # write (bias already folded into E)
                                    nc.scalar.activation(
                                        ot[:, :, pa, :, pw],
                                        t1[:, lo : lo + 16, :],
                                        IDENT,
                                        bias=0.0,
                                        scale=1.0,
                                    )
                            nc.sync.dma_start(
                                out_d[cob, :, hb2, :],
                                ot[:].rearrange("c a b w v -> c (a b w v)"),
                            )
    return nc


_CACHED_NC = {}


def _get_nc(reps: int = 1) -> bass.Bass:
    if reps not in _CACHED_NC:
        _CACHED_NC[reps] = build_nc(reps)
    return _CACHED_NC[reps]


def _prepare(x, weight, bias, reps: int = 1):
    Wmat = _stage1_weight_matrix(np.asarray(weight, dtype=np.float32)).astype(
        ml_dtypes.bfloat16
    )
    b2 = np.ascontiguousarray(
        np.asarray(bias, dtype=np.float32).reshape(2, 128) / np.float32(8.0)
    )
    xs = np.pad(
        np.asarray(x, dtype=np.float32), ((0, 0), (0, 0), (1, 1), (1, 1))
    ).astype(ml_dtypes.bfloat16)
    nc = _get_nc(reps)
    in_maps = [{"x": xs[i], "w": Wmat, "bias8": b2} for i in range(N_CORES)]
    return in_maps, nc


def _run(x, weight, bias, reps: int = 1):
    in_maps, nc = _prepare(x, weight, bias, reps)
    res = run_bass_kernel_spmd(nc, in_maps, list(range(N_CORES)))
    return np.stack(
        [res.results[i]["out"].reshape(C, OH, OW) for i in range(N_CORES)]
    )


def kernel(x, weight, bias):
    return _run(x, weight, bias, reps=1)


# revision 25
# speedup vs baseline: 1.0210x; 1.0114x over previous
"""Trainium2 Bass kernel for upsample_conv_2d (conv_transpose stride-2 3x3 +
4x4 FIR + bias), data-parallel over batch on 8 NeuronCores.

Formulation (2x less PE work than composing conv+FIR into one 6x6 kernel,
and vector work small enough to hide under the PE):

stage 1: The conv_transpose phase images combined with the W-DIRECTION half
of the separable 4x4 FIR (fir = outer([1,3,3,1],[1,3,3,1])/16) collapse into
18 composed channel-contraction taps: for output column parity pw and row
set a (a=0: even y-rows from w rows p=0,2; a=1: odd y-rows from p=1),

    E[a,pw][i,S] = sum_{d,o} CW[a,pw,d,o] . xp[i+d, S+o]   (+ bias/8)

with CW 3 column-taps (o) per row-tap (d); a=0 has d in {0,1}, a=1 only d=1.
All taps run as bf16 matmuls accumulating in fp32 PSUM; the PSUM drain adds
bias/8 (so the 8-coefficient pass-h sum reconstructs + bias exactly) and
writes bf16 E tiles. E[a=1] is stored with pad rows 0,65 holding bias/8.

stage 2 (pass-h only): out rows by parity pa combine two E images with
integer weights (1/16 already folded into CW):

    out[2R+pa, *] = 3*E0[R+pa] + E0[R+1-pa] + 3*F1[R+1] + F1[R+2*pa]

as two whole-image scalar_tensor_tensor ops (DVE, bf16 aligned fast path)
plus one tensor_tensor add per 16-row block writing the strided f32 output
tile directly (split DVE/GpSimd). Output DMA moves flat [128 x 16KB]
contiguous blocks (out DRAM tensor declared layout-equivalent
[2,128,4,4096]).
"""

import json

import numpy as np
import ml_dtypes

import concourse.bass as bass
import concourse.mybir as mybir
import concourse.tile as tile
from concourse.bass_utils import run_bass_kernel_spmd

# ---------------------------------------------------------------------------
# BIR post-pass: this walrus build rejects instructions carrying more than one
# sem wait (e.g. Tile's kernel-tail Drain gets 3). Hoist extras into
# standalone EventSemaphore instructions right before the owner.
# ---------------------------------------------------------------------------
_MAX_WAITS = 1


def _split_waits(j: dict) -> dict:
    for fn in j.get("functions", []):
        for blk in fn.get("blocks", []):
            insts = blk.get("instructions")
            if not insts:
                continue
            out = []
            for inst in insts:
                si = inst.get("sync_info") or {}
                waits = si.get("on_wait") or []
                if len(waits) > _MAX_WAITS:
                    for k, w in enumerate(waits[_MAX_WAITS:]):
                        out.append(
                            {
                                "debug": inst.get("debug", 0),
                                "engine": inst["engine"],
                                "ins": [],
                                "name": f"{inst['name']}-wsplit{k}",
                                "opcode": "EventSemaphore",
                                "outs": [],
                                "sync_info": {"on_update": [], "on_wait": [w]},
                            }
                        )
                    si["on_wait"] = waits[:_MAX_WAITS]
                out.append(inst)
            blk["instructions"] = out
    return j


_orig_to_json_bytes = bass.Bass.to_json_bytes


def _patched_to_json_bytes(self):
    return json.dumps(_split_waits(json.loads(_orig_to_json_bytes(self)))).encode()


bass.Bass.to_json_bytes = _patched_to_json_bytes

# ---------------------------------------------------------------------------
# Problem constants (hardcoded; kernel.py must be self-contained)
# ---------------------------------------------------------------------------
N, C, H, W = 8, 256, 64, 64
OH, OW = 2 * H, 2 * W
N_CORES = 8
F32 = mybir.dt.float32
BF16 = mybir.dt.bfloat16
AOP = mybir.AluOpType
IDENT = mybir.ActivationFunctionType.Identity

# stage-1 composed tap schedule: per pw, rows (a=0,d=0,p=0), (a=0,d=1,p=2),
# (a=1,d=1,p=1), each with col offsets o=0,1,2. t = pw*9 + row*3 + o.
_ROWS = ((0, 0, 0), (0, 1, 2), (1, 1, 1))  # (a, d, p)
_NTAP = 18


def _stage1_weight_matrix(w: np.ndarray) -> np.ndarray:
    """[256,256,3,3] weight -> [128, 72*128] lhsT matrix (f32, cast later).

    Column block index = (t*2 + cib)*2 + cob; block = CW[t][cob128, cib128].T.
    CW composes the w-direction FIR half into the conv taps, including the
    1/16 FIR normalization.
    """
    Wq = {(p, q): w[:, :, p, q].astype(np.float64) for p in range(3) for q in range(3)}
    mats = []
    for pw in range(2):
        for a, d, p in _ROWS:
            if pw == 0:
                cc = {
                    0: 3 * Wq[p, 0] + Wq[p, 1],
                    1: 3 * Wq[p, 2] + Wq[p, 0] + 3 * Wq[p, 1],
                    2: Wq[p, 2],
                }
            else:
                cc = {
                    0: Wq[p, 0],
                    1: 3 * Wq[p, 0] + Wq[p, 2] + 3 * Wq[p, 1],
                    2: 3 * Wq[p, 2] + Wq[p, 1],
                }
            for o in range(3):
                mats.append(cc[o] / 16.0)
    Wmat = np.zeros((128, _NTAP * 4, 128), dtype=np.float32)
    for t, M in enumerate(mats):
        for cib in range(2):
            for cob in range(2):
                blk = M[cob * 128 : (cob + 1) * 128, cib * 128 : (cib + 1) * 128]
                Wmat[:, (t * 2 + cib) * 2 + cob, :] = blk.T.astype(np.float32)
    return Wmat.reshape(128, -1)


def _tap_index(t: int, cib: int, cob: int) -> int:
    return (t * 2 + cib) * 2 + cob


def _blocks(nrows: int):
    out = []
    r = 0
    while r < nrows:
        nr = min(8, nrows - r)  # 8 rows x 64 = 512 fp32 = exactly one PSUM bank
        out.append((r, nr))
        r += nr
    return out


def build_nc(reps: int = 1) -> bass.Bass:
    nc = bass.Bass("TRN2", target_bir_lowering=False, debug=False)
    x_d = nc.dram_tensor("x", [C, H + 2, W + 2], BF16, kind="ExternalInput").ap()
    w_d = nc.dram_tensor("w", [128, _NTAP * 4 * 128], BF16, kind="ExternalInput").ap()
    b_d = nc.dram_tensor("bias8", [2, 128], F32, kind="ExternalInput").ap()
    # layout-equivalent view of [C, OH, OW]: [cob, ch, hb2, 32*OW]
    out_d = nc.dram_tensor("out", [2, 128, 4, 32 * OW], F32, kind="ExternalOutput").ap()

    xb = x_d.rearrange("(b p) h w -> b p h w", p=128)

    with tile.TileContext(nc) as tc:
        with (
            tc.tile_pool(name="weights", bufs=1) as wpool,
            tc.tile_pool(name="xin", bufs=1) as xpool,
            tc.tile_pool(name="ebuf", bufs=1) as epool,
            tc.tile_pool(name="psum", bufs=8, space="PSUM") as ppool,
            tc.tile_pool(name="outs", bufs=2) as opool,
            tc.tile_pool(name="scratch", bufs=2) as spool,
        ):
            wt = wpool.tile([128, _NTAP * 4, 128], BF16)
            nc.sync.dma_start(wt[:], w_d.rearrange("p (a b) -> p a b", b=128))
            bt8 = wpool.tile([128, 2], F32)
            nc.sync.dma_start(bt8[:], b_d.rearrange("b p -> p b"))

            xpad = [
                xpool.tile([128, H + 2, W + 2], BF16, tag=f"xp{i}", name=f"xp{i}")
                for i in range(2)
            ]
            for cib in range(2):
                nc.sync.dma_start(xpad[cib][:], xb[cib])

            # E tiles: E0 [65,64]; F1 [66,64] with pad rows 0,65 = bias/8
            te = {}
            for cob in range(2):
                for pw in range(2):
                    te[0, pw, cob] = epool.tile(
                        [128, 65, 64], BF16, tag=f"e0{pw}_{cob}", name=f"e0{pw}_{cob}"
                    )
                    te[1, pw, cob] = epool.tile(
                        [128, 66, 64], BF16, tag=f"e1{pw}_{cob}", name=f"e1{pw}_{cob}"
                    )
            # fill F1 pad rows with bias/8 once (out = in*0 + bias)
            for cob in range(2):
                for pw in range(2):
                    for rr in (0, 65):
                        nc.scalar.activation(
                            te[1, pw, cob][:, rr : rr + 1, :],
                            xpad[0][:, 0:1, 0:64],
                            IDENT,
                            bias=bt8[:, cob : cob + 1],
                            scale=0.0,
                        )

            def drain(dst_ap, psum_ap, cob):
                # ACT only: keeps DVE free for the pass-h stt ops
                nc.scalar.activation(
                    dst_ap, psum_ap, IDENT, bias=bt8[:, cob : cob + 1], scale=1.0
                )

            fcnt = 0
            for _rep in range(reps):
                for cob in range(2):
                    # ---- stage 1: E images via composed-tap PE matmuls ----
                    for pw in range(2):
                        for ri, (a, d, _p) in enumerate(_ROWS):
                            et = te[a, pw, cob]
                            nrows = 65 if a == 0 else 64
                            # a=0 accumulates rows (0,0,0) and (0,1,2) into one
                            # psum pass; handled below by grouping on a.
                            if ri == 1:
                                continue  # merged into ri == 0 pass
                            if a == 0:
                                row_list = [_ROWS[0], _ROWS[1]]
                            else:
                                row_list = [_ROWS[2]]
                            for r0, nr in _blocks(nrows):
                                ps = ppool.tile([128, nr, 64], F32, tag="ps", name="ps")
                                nmm = len(row_list) * 3 * 2
                                i = 0
                                for rj, (aa, dd, _pp) in enumerate(row_list):
                                    tbase = pw * 9 + (_ROWS.index((aa, dd, _pp))) * 3
                                    for o in range(3):
                                        for cib in range(2):
                                            lhsT = wt[:, _tap_index(tbase + o, cib, cob), :]
                                            rhs = xpad[cib][
                                                :, r0 + dd : r0 + dd + nr, o : o + 64
                                            ]
                                            nc.tensor.matmul(
                                                ps[:],
                                                lhsT,
                                                rhs,
                                                start=(i == 0),
                                                stop=(i == nmm - 1),
                                            )
                                            i += 1
                                dst = (
                                    et[:, r0 : r0 + nr, :]
                                    if a == 0
                                    else et[:, r0 + 1 : r0 + 1 + nr, :]
                                )
                                drain(dst, ps[:], cob)

                    # ---- stage 2: pass-h in 32-R-row halves ----
                    for half in range(2):
                        r0 = half * 32
                        tsc = {}
                        for pw in range(2):
                            e0 = te[0, pw, cob]
                            f1 = te[1, pw, cob]
                            for pa in range(2):
                                t1 = spool.tile(
                                    [128, 32, 64], BF16,
                                    tag=f"t1_{pw}{pa}", name="t1",
                                )
                                t2 = spool.tile(
                                    [128, 32, 64], BF16,
                                    tag=f"t2_{pw}{pa}", name="t2",
                                )
                                e0_3 = e0[:, r0 + pa : r0 + pa + 32, :]
                                e0_1 = e0[:, r0 + 1 - pa : r0 + 33 - pa, :]
                                f1_3 = f1[:, r0 + 1 : r0 + 33, :]
                                f1_1 = f1[:, r0 + 2 * pa : r0 + 2 * pa + 32, :]
                                nc.vector.scalar_tensor_tensor(
                                    t1[:], e0_3, 3.0, e0_1, AOP.mult, AOP.add
                                )
                                nc.vector.scalar_tensor_tensor(
                                    t2[:], f1_3, 3.0, f1_1, AOP.mult, AOP.add
                                )
                                # sum into t1 in place on GpSimd (contiguous
                                # bf16; aliasing is free)
                                nc.gpsimd.tensor_tensor(
                                    t1[:], t1[:], t2[:], AOP.add
                                )
                                tsc[pw, pa] = t1
                        for sub in range(2):
                            hb2 = half * 2 + sub
                            lo = sub * 16
                            ot = opool.tile(
                                [128, 16, 2, 64, 2], F32, tag="ot", name="ot"
                            )
                            for pw in range(2):
                                for pa in range(2):
                                    t1 = tsc[pw, pa]
                                    # ACT does the strided f32 interleave
                                    # write (bias already folded into E)
                                    nc.scalar.activation(
                                        ot[:, :, pa, :, pw],
                                        t1[:, lo : lo + 16, :],
                                        IDENT,
                                        bias=0.0,
                                        scale=1.0,
                                    )
                            nc.sync.dma_start(
                                out_d[cob, :, hb2, :],
                                ot[:].rearrange("c a b w v -> c (a b w v)"),
                            )
    return nc


_CACHED_NC = {}


def _get_nc(reps: int = 1) -> bass.Bass:
    if reps not in _CACHED_NC:
        _CACHED_NC[reps] = build_nc(reps)
    return _CACHED_NC[reps]


def _prepare(x, weight, bias, reps: int = 1):
    Wmat = _stage1_weight_matrix(np.asarray(weight, dtype=np.float32)).astype(
        ml_dtypes.bfloat16
    )
    b2 = np.ascontiguousarray(
        np.asarray(bias, dtype=np.float32).reshape(2, 128) / np.float32(8.0)
    )
    xs = np.pad(
        np.asarray(x, dtype=np.float32), ((0, 0), (0, 0), (1, 1), (1, 1))
    ).astype(ml_dtypes.bfloat16)
    nc = _get_nc(reps)
    in_maps = [{"x": xs[i], "w": Wmat, "bias8": b2} for i in range(N_CORES)]
    return in_maps, nc


def _run(x, weight, bias, reps: int = 1):
    in_maps, nc = _prepare(x, weight, bias, reps)
    res = run_bass_kernel_spmd(nc, in_maps, list(range(N_CORES)))
    return np.stack(
        [res.results[i]["out"].reshape(C, OH, OW) for i in range(N_CORES)]
    )


def kernel(x, weight, bias):
    return _run(x, weight, bias, reps=1)
